# revision 1
# baseline (speedup 1.0000x reference)
"""Self-contained 8-core Trainium2 Bass kernel for nn_MultiHeadAttention.

Sharding: core c = (b, g), b = c // 4 (batch), g = c % 4 (kv head group).
Each core computes heads 4g..4g+3 for batch b (they share kv head g),
produces a partial [S, M] output through its Wo row-slice; the host sums
the 4 group partials per batch.
"""
import numpy as np
import ml_dtypes

import concourse.bass as bass
import concourse.mybir as mybir
import concourse.tile as tile
from concourse import bass_utils

F32 = mybir.dt.float32
BF16 = mybir.dt.bfloat16
ALU = mybir.AluOpType
ACT = mybir.ActivationFunctionType

B, S, M, H, HKV, D = 2, 2048, 1024, 16, 4, 64
HL = H // HKV          # local q heads per core = 4
PI = float(np.pi)
TWO_PI = float(2 * np.pi)


def _split_sync_waits(nc, limit=1):
    """This container's walrus rejects >1 sync-wait per instruction; move
    excess waits onto same-engine NOPs inserted just before."""
    ctr = 0
    for f in nc.m.functions:
        for bb in f.blocks:
            il = bb.instructions
            i = 0
            while i < len(il):
                inst = il[i]
                si = getattr(inst, "sync_info", None)
                if si is None:
                    i += 1
                    continue
                waits = list(si.on_wait)
                if len(waits) <= limit:
                    i += 1
                    continue
                keep, rest = waits[:limit], waits[limit:]
                nops = []
                for j in range(0, len(rest), limit):
                    ctr += 1
                    nop = mybir.InstNoOp(name=f"I-wsplit-{ctr}", ins=[], outs=[])
                    nop.engine = inst.engine
                    nop.sync_info = mybir.SyncInfo(
                        on_update=[], on_wait=rest[j:j + limit])
                    nops.append(nop)
                si.on_wait = keep
                inst.sync_info = si
                for k, nop in enumerate(nops):
                    il.insert(i + k, nop)
                i += len(nops) + 1
            bb.instructions = il


def emit_mha(nc, tc, s_len=S, chunk=512, kb=3, reps=1):
    """Emit the per-core MHA kernel body. s_len tokens, q-chunks of
    `chunk`, exp batches of `kb` k-tiles. reps>1 re-emits the body for
    wall-clock delta timing."""
    T = s_len // 128           # s-tiles
    MT = M // 128              # m-tiles of the model dim
    NJ = s_len // chunk        # q chunks
    HD = HL * D                # 256

    xqt = nc.declare_dram_parameter("xqt", [M, s_len], BF16, isOutput=False)
    wqkv = nc.declare_dram_parameter("wqkv", [M, HD + 2 * D], BF16, isOutput=False)
    wo = nc.declare_dram_parameter("wo", [HD, M], BF16, isOutput=False)
    qpos = nc.declare_dram_parameter("qpos", [128, 2 * T], F32, isOutput=False)
    kpos = nc.declare_dram_parameter("kpos", [128, 2 * T], F32, isOutput=False)
    invf = nc.declare_dram_parameter("invf", [128, 16], F32, isOutput=False)
    iden = nc.declare_dram_parameter("iden", [128, 128], BF16, isOutput=False)
    out = nc.declare_dram_parameter("out", [s_len, M], F32, isOutput=True)

    for _ in range(reps):
        _emit_body(nc, tc, s_len, chunk, kb, T, MT, NJ, HD,
                   xqt, wqkv, wo, qpos, kpos, invf, iden, out)


def _emit_body(nc, tc, s_len, chunk, kb, T, MT, NJ, HD,
               xqt, wqkv, wo, qpos, kpos, invf, iden, out):
    with tc.tile_pool(name="persist", bufs=1) as pp:
        # ---- persistent SBUF ----
        xqt_sb = pp.tile([128, MT, s_len], BF16, tag="xqt")
        wqkv_sb = pp.tile([128, MT, HD + 2 * D], BF16, tag="wqkv")
        wo_sb = pp.tile([128, HD // 128, M], BF16, tag="wo")
        qpos_sb = pp.tile([128, T, 2], F32, tag="qpos")
        kpos_sb = pp.tile([128, T, 2], F32, tag="kpos")
        invf_sb = pp.tile([128, 16], F32, tag="invf")
        iden_sb = pp.tile([128, 128], BF16, tag="iden")

        nc.sync.dma_start(qpos_sb[:], qpos.rearrange("p (t c) -> p t c", c=2))
        nc.sync.dma_start(kpos_sb[:], kpos.rearrange("p (t c) -> p t c", c=2))
        nc.sync.dma_start(invf_sb[:], invf[:])
        nc.sync.dma_start(iden_sb[:], iden[:])
        nc.sync.dma_start(wqkv_sb[:], wqkv.rearrange("(mt p) n -> p mt n", p=128))
        nc.sync.dma_start(wo_sb[:], wo.rearrange("(k p) n -> p k n", p=128))
        xqt_r = xqt.rearrange("(mt p) s -> p mt s", p=128)
        sq_sz = 512 if s_len % 512 == 0 else s_len
        for q0 in range(0, s_len, sq_sz):
            nc.sync.dma_start(xqt_sb[:, :, q0:q0 + sq_sz],
                              xqt_r[:, :, q0:q0 + sq_sz])

        # constants
        ones64 = pp.tile([128, 64], BF16, tag="ones64")
        nc.vector.memset(ones64[:], 1.0)

        # ---- rope tables: cos/sin for q and k, [128, T, 2, 16] bf16 ----
        tabs = {}
        with tc.tile_pool(name="tabtmp", bufs=2) as tp:
            for nm, pos_sb in (("q", qpos_sb), ("k", kpos_sb)):
                freq = tp.tile([128, T * 32], F32, tag="freq")
                nc.vector.tensor_tensor(
                    freq[:].rearrange("p (t c f) -> p t c f", c=2, f=16),
                    pos_sb[:].unsqueeze(3).broadcast_to((128, T, 2, 16)),
                    invf_sb[:].unsqueeze(1).unsqueeze(1)
                    .broadcast_to((128, T, 2, 16)),
                    ALU.mult)
                sarg = tp.tile([128, T * 32], F32, tag="sarg")
                carg = tp.tile([128, T * 32], F32, tag="carg")
                ge = tp.tile([128, T * 32], F32, tag="ge")
                yi = tp.tile([128, T * 32], mybir.dt.int32, tag="yi")
                yf = tp.tile([128, T * 32], F32, tag="yf")
                # m = freq - 2pi*int(freq/2pi)  (freq >= 0)
                nc.vector.tensor_scalar(yf[:], freq[:], 1.0 / TWO_PI, None,
                                        op0=ALU.mult)
                nc.vector.tensor_copy(yi[:], yf[:])
                nc.vector.tensor_copy(yf[:], yi[:])
                m = freq
                nc.vector.scalar_tensor_tensor(m[:], yf[:], -TWO_PI, freq[:],
                                               op0=ALU.mult, op1=ALU.add)
                # sarg = wrap(m) into [-pi, pi]
                nc.vector.tensor_scalar(ge[:], m[:], PI, None, op0=ALU.is_gt)
                nc.vector.scalar_tensor_tensor(sarg[:], ge[:], -TWO_PI, m[:],
                                               op0=ALU.mult, op1=ALU.add)
                # carg = wrap(m + pi/2)
                nc.vector.tensor_scalar(carg[:], m[:], PI / 2, None, op0=ALU.add)
                nc.vector.tensor_scalar(ge[:], carg[:], PI, None, op0=ALU.is_gt)
                nc.vector.scalar_tensor_tensor(carg[:], ge[:], -TWO_PI, carg[:],
                                               op0=ALU.mult, op1=ALU.add)
                sin_t = pp.tile([128, T * 32], BF16, tag=f"sin_{nm}")
                cos_t = pp.tile([128, T * 32], BF16, tag=f"cos_{nm}")
                nc.scalar.activation(sin_t[:], sarg[:], ACT.Sin)
                nc.scalar.activation(cos_t[:], carg[:], ACT.Sin)
                tabs[nm] = (cos_t, sin_t)

        # ---- projection + ssq ----
        qkv_sb = [pp.tile([128, 6, 64], F32, tag=f"qkv{t}", name=f"qkv{t}")
                  for t in range(T)]
        allssq = pp.tile([128, T, 6], F32, tag="allssq")
        invrms = pp.tile([128, T, 6], F32, tag="invrms")
        epsb = pp.tile([128, 1], F32, tag="epsb")
        nc.vector.memset(epsb[:], 1e-6)
        with tc.tile_pool(name="psum_proj", bufs=2, space="PSUM") as prp, \
             tc.tile_pool(name="sqtmp", bufs=2) as sqp:
            for t in range(T):
                ps = prp.tile([128, HD + 2 * D], F32, tag="proj")
                for m in range(MT):
                    nc.tensor.matmul(
                        ps[:], xqt_sb[:, m, t * 128:(t + 1) * 128],
                        wqkv_sb[:, m, :],
                        start=(m == 0), stop=(m == MT - 1))
                nc.any.tensor_copy(
                    qkv_sb[t][:], ps[:].rearrange("p (h d) -> p h d", d=64))
                sq = sqp.tile([128, 6, 64], F32, tag="sq")
                nc.vector.tensor_tensor(sq[:], qkv_sb[t][:], qkv_sb[t][:],
                                        ALU.mult)
                nc.vector.tensor_reduce(
                    allssq[:, t:t + 1, :].rearrange("p a b -> p (a b)"),
                    sq[:], axis=mybir.AxisListType.X, op=ALU.add)
                # invrms = rsqrt(ssq/64 + eps) per half, to unblock rope early
                if t == T // 2 - 1 or t == T - 1:
                    lo = 0 if t < T // 2 else T // 2
                    sl = (slice(None), slice(lo, t + 1), slice(None))
                    nc.scalar.activation(invrms[sl], allssq[sl], ACT.Ln,
                                         scale=1.0 / 64.0, bias=epsb[:])
                    nc.scalar.activation(invrms[sl], invrms[sl], ACT.Exp,
                                         scale=-0.5)
                    nc.vector.memset(invrms[:, lo:t + 1, 5:6], 1.0)

        # ---- norm + rope + transpose ----
        qt_sb = [pp.tile([128, s_len], BF16, tag=f"qt{h}", name=f"qt{h}")
                 for h in range(HL)]
        kt_sb = pp.tile([128, s_len], BF16, tag="kt")
        vb = [pp.tile([128, 64], BF16, tag=f"v{t}", name=f"v{t}") for t in range(T)]
        (cq, sq), (ck, sk) = tabs["q"], tabs["k"]
        with tc.tile_pool(name="rope", bufs=3) as rp, \
             tc.tile_pool(name="psum_tr", bufs=4, space="PSUM") as trp:
            for t in range(T):
                qkvbf = rp.tile([128, 6, 64], BF16, tag="qkvbf")
                nc.vector.tensor_tensor(
                    qkvbf[:], qkv_sb[t][:],
                    invrms[:, t:t + 1, :].rearrange("p a b -> p (a b)")
                    .unsqueeze(2).broadcast_to((128, 6, 64)),
                    ALU.mult)
                nc.any.tensor_copy(vb[t][:], qkvbf[:, 5:6, :].squeeze(1))
                qro = rp.tile([128, 5, 64], BF16, tag="qro")
                tmp1 = rp.tile([128, 128], BF16, tag="tmp1")
                tmp2 = rp.tile([128, 128], BF16, tag="tmp2")
                for nm, h0, nh, (cos_t, sin_t) in (
                        ("q", 0, HL, (cq, sq)), ("k", HL, 1, (ck, sk))):
                    fl = qkvbf[:, h0:h0 + nh, :].rearrange(
                        "p h (c u f) -> p h c u f", c=2, u=2)
                    a1 = fl[:, :, :, 0:1, :].squeeze(3)
                    a2 = fl[:, :, :, 1:2, :].squeeze(3)
                    ro = qro[:, h0:h0 + nh, :].rearrange(
                        "p h (c u f) -> p h c u f", c=2, u=2)
                    o1 = ro[:, :, :, 0:1, :].squeeze(3)
                    o2 = ro[:, :, :, 1:2, :].squeeze(3)
                    cosv = cos_t[:, t * 32:(t + 1) * 32] \
                        .rearrange("p (c f) -> p c f", f=16).unsqueeze(1) \
                        .broadcast_to((128, nh, 2, 16))
                    sinv = sin_t[:, t * 32:(t + 1) * 32] \
                        .rearrange("p (c f) -> p c f", f=16).unsqueeze(1) \
                        .broadcast_to((128, nh, 2, 16))
                    w1 = tmp1[:, 0:nh * 32].rearrange(
                        "p (h c f) -> p h c f", c=2, f=16)
                    w2 = tmp2[:, 0:nh * 32].rearrange(
                        "p (h c f) -> p h c f", c=2, f=16)
                    nc.vector.tensor_tensor(w1, a1, cosv, ALU.mult)
                    nc.vector.tensor_tensor(w2, a2, sinv, ALU.mult)
                    nc.vector.tensor_tensor(o1, w1, w2, ALU.subtract)
                    nc.vector.tensor_tensor(w1, a2, cosv, ALU.mult)
                    nc.vector.tensor_tensor(w2, a1, sinv, ALU.mult)
                    nc.vector.tensor_tensor(o2, w1, w2, ALU.add)
                for h in range(HL + 1):
                    dst = kt_sb if h == HL else qt_sb[h]
                    pt = trp.tile([64, 128], BF16, tag="tr")
                    nc.tensor.transpose(
                        pt[:], qro[:, h:h + 1, :].squeeze(1), iden_sb[:])
                    nc.any.tensor_copy(
                        dst[0:64, t * 128:(t + 1) * 128], pt[:])
        # duplicate to partitions 64:128 for row-group packing
        for h in range(HL):
            nc.vector.tensor_copy(qt_sb[h][64:128, :], qt_sb[h][0:64, :])
        nc.vector.tensor_copy(kt_sb[64:128, :], kt_sb[0:64, :])

        # ---- attention ----
        out_t = [pp.tile([128, s_len], BF16, tag=f"outT{hp}", name=f"outT{hp}")
                 for hp in range(HL // 2)]
        kts = list(range(T))
        batches = [kts[i:i + kb] for i in range(0, T, kb)]
        with tc.tile_pool(name="sc", bufs=2, space="PSUM") as scp, \
             tc.tile_pool(name="av", bufs=1, space="PSUM") as avp, \
             tc.tile_pool(name="se", bufs=1, space="PSUM") as sep, \
             tc.tile_pool(name="expt", bufs=4) as ep, \
             tc.tile_pool(name="smtmp", bufs=2) as smp:
            for j in range(NJ):
                for hp in range(HL // 2):
                    se = sep.tile([128, chunk], F32, tag="se")
                    avt = avp.tile([128, chunk], F32, tag="av")
                    expts = {}
                    for bi, batch in enumerate(batches):
                        for hh in range(2):
                            h = 2 * hp + hh
                            sc = scp.tile([128, kb * chunk], F32, tag="sc")
                            for ki, kt in enumerate(batch):
                                rg = kt % 2
                                nc.tensor.matmul(
                                    sc[:, ki * chunk:(ki + 1) * chunk],
                                    kt_sb[rg * 64:(rg + 1) * 64,
                                          kt * 128:(kt + 1) * 128],
                                    qt_sb[h][rg * 64:(rg + 1) * 64,
                                             j * chunk:(j + 1) * chunk],
                                    start=True, stop=True,
                                    tile_position=(rg * 64, 0))
                            et = ep.tile([128, kb * chunk], BF16, tag="expt")
                            nc.scalar.activation(
                                et[:, 0:len(batch) * chunk],
                                sc[:, 0:len(batch) * chunk],
                                ACT.Exp, scale=0.125)
                            expts[hh] = et
                        for ki, kt in enumerate(batch):
                            for hh in range(2):
                                h = 2 * hp + hh
                                nc.tensor.matmul(
                                    avt[hh * 64:(hh + 1) * 64, :],
                                    vb[kt][:],
                                    expts[hh][:, ki * chunk:(ki + 1) * chunk],
                                    start=(kt == 0), stop=(kt == T - 1),
                                    tile_position=(0, hh * 64),
                                    skip_group_check=True)
                                nc.tensor.matmul(
                                    se[hh * 64:(hh + 1) * 64, :],
                                    ones64[:],
                                    expts[hh][:, ki * chunk:(ki + 1) * chunk],
                                    start=(kt == 0), stop=(kt == T - 1),
                                    tile_position=(0, hh * 64),
                                    skip_group_check=True)
                    # 1/sumexp via exp(-ln(x)); se rows already replicated
                    # across each head's 64 partitions
                    rec = smp.tile([128, chunk], F32, tag="rec")
                    nc.scalar.activation(rec[:], se[:], ACT.Ln)
                    nc.scalar.activation(rec[:], rec[:], ACT.Exp, scale=-1.0)
                    nc.vector.tensor_tensor(
                        out_t[hp][:, j * chunk:(j + 1) * chunk],
                        avt[:], rec[:], ALU.mult)

        # ---- O-projection ----
        with tc.tile_pool(name="psum_o", bufs=4, space="PSUM") as pop, \
             tc.tile_pool(name="ostage", bufs=3) as osp:
            for t in range(T):
                ost = osp.tile([128, M], F32, tag="ost")
                for n in range(M // 512):
                    po = pop.tile([128, 512], F32, tag="po")
                    for k in range(HD // 128):
                        nc.tensor.matmul(
                            po[:], out_t[k][:, t * 128:(t + 1) * 128],
                            wo_sb[:, k, n * 512:(n + 1) * 512],
                            start=(k == 0), stop=(k == HD // 128 - 1))
                    nc.any.tensor_copy(ost[:, n * 512:(n + 1) * 512], po[:])
                nc.sync.dma_start(out[t * 128:(t + 1) * 128, :], ost[:])


_NC_CACHE = {}


def _build(s_len=S, chunk=512, kb=3, reps=1):
    key = (s_len, chunk, kb, reps)
    if key not in _NC_CACHE:
        nc = bass.Bass()
        with tile.TileContext(nc) as tc:
            emit_mha(nc, tc, s_len=s_len, chunk=chunk, kb=kb, reps=reps)
        _split_sync_waits(nc)
        _NC_CACHE[key] = nc
    return _NC_CACHE[key]


def _prep_core_inputs(x_q, q_pos, k_pos, Wq, Wk, Wv, Wo, b, g, s_len=S):
    T = s_len // 128
    bf = ml_dtypes.bfloat16
    xqt = np.ascontiguousarray(x_q[b].T).astype(bf)
    wqkv = np.concatenate(
        [Wq[:, 4 * g:4 * g + 4, :].reshape(M, HL * D),
         Wk[:, g, :], Wv[:, g, :]], axis=1).astype(bf)
    wo = Wo[HL * D * g:HL * D * (g + 1), :].astype(bf)
    qp = q_pos[b].astype(np.float32).reshape(T, 128, 2) \
        .transpose(1, 0, 2).reshape(128, 2 * T)
    kp = k_pos[b].astype(np.float32).reshape(T, 128, 2) \
        .transpose(1, 0, 2).reshape(128, 2 * T)
    invf = (10000.0 ** (-np.arange(0, 32, 2, dtype=np.float32) / 32.0))
    invf = np.broadcast_to(invf[None, :], (128, 16)).copy()
    iden = np.eye(128, dtype=bf)
    return {"xqt": np.ascontiguousarray(xqt),
            "wqkv": np.ascontiguousarray(wqkv),
            "wo": np.ascontiguousarray(wo),
            "qpos": np.ascontiguousarray(qp),
            "kpos": np.ascontiguousarray(kp),
            "invf": invf, "iden": iden}


def kernel(x_q, q_pos, k_pos, Wq, Wk, Wv, Wo):
    x_q, q_pos, k_pos = np.asarray(x_q), np.asarray(q_pos), np.asarray(k_pos)
    Wq, Wk, Wv, Wo = (np.asarray(w) for w in (Wq, Wk, Wv, Wo))
    nc = _build()
    in_maps = [
        _prep_core_inputs(x_q, q_pos, k_pos, Wq, Wk, Wv, Wo, c // 4, c % 4)
        for c in range(8)]
    res = bass_utils.run_bass_kernel_spmd(nc, in_maps, core_ids=list(range(8)))
    out = np.zeros((B, S, M), np.float32)
    for c in range(8):
        out[c // 4] += np.asarray(res.results[c]["out"], dtype=np.float32)
    return out

